# revision 23
# baseline (speedup 1.0000x reference)
"""Trainium2 Bass kernel for nn_LiveNet (2-layer MLP: relu(x@W1+b1)@W2+b2).

Sharding: pure data-parallel over batch across 8 NeuronCores (no
collectives).  Each core computes y_i = relu(x_i @ W1 + b1) @ W2 + b2 for
its 512-row batch shard.

Numerics: split-fp8 DoubleRow.  Every matmul operand is decomposed as
a = q8(s*a) + q8(s*a - q8(s*a)) (e4m3 hi + e4m3 residual, scale s chosen
so values sit in e4m3's normal range).  Each GEMM is then computed as
hi@Hi + hi@Lo + lo@Hi (the lo@lo term is ~0.07% and dropped), with all
three terms at the SAME power-of-two psum scale so they accumulate into
one PSUM bank.  fp8e4 matmuls run in MatmulPerfMode.DoubleRow (K=256 per
instruction at 0.5 cyc/row), so the 3-term scheme costs 0.75x of a bf16
GEMM while matching bf16 accuracy (~0.2% rel).

Dataflow per core:
  GEMM1: hT tiles [hid,batch]; lhsT = W1 hi/lo chunks (stationary), rhs =
         x hi/lo planes (moving, N=512).  12 DR matmuls per m-tile into
         one PSUM bank, with the xl-correction pass deferred by DEFER
         tiles so the xl DMA streams under the hi-term compute; ACT evicts
         relu(ps/32 + b1) -> bf16, DVE requantizes to (hh, hl) fp8 pairs.
  GEMM2: y[batch,out] bank-serial (k innermost) so output banks finish
         (and stream out) one at a time instead of all at the end.  DVE
         pre-seeds each bank with 64*b2, 48 DR matmuls accumulate with
         start=False, and the evict is a single ACT copy (scale 1/64).
         The last bank is split [384|128] cols so its writeout chain
         mostly hides under the 128-col piece's compute.

All fp8 inputs are packed on host into one consumption-ordered DRAM blob
streamed just-in-time by a hand-paced DMA schedule; a single tiny warmup
matmul at t~0 locks the PE p-state at full clock (pe_busy_start never
resets), so all real matmuls run at 2.4 GHz from the first instruction.
"""

import os
import sys

import numpy as np
import ml_dtypes

for _p in ("/opt/trn_rl_repo", "/root/.axon_site/_ro/trn_rl_repo"):
    if os.path.isdir(_p) and _p not in sys.path:
        sys.path.append(_p)

import concourse.bacc as bacc
import concourse.bass as bass
import concourse.tile as tile
from concourse import mybir
from concourse.bass_utils import run_bass_kernel_spmd

N_CORES = 8
B, N_IN, N_HID, N_OUT = 4096, 1024, 4096, 1024
BSH = B // N_CORES          # 512 batch rows per core
P = 128                     # SBUF partitions
J1 = N_IN // 256            # 4  DoubleRow K-groups in GEMM1
MT1 = N_HID // P            # 32 m-tiles (hid) in GEMM1
KK2 = N_HID // 256          # 16 DoubleRow K-groups in GEMM2
MB2 = BSH // P              # 4  batch tiles in GEMM2
NCH = 512                   # psum moving width
NT2 = N_OUT // NCH          # 2  out chunks in GEMM2

F32 = mybir.dt.float32
BF16 = mybir.dt.bfloat16
F8 = mybir.dt.float8e4
E4 = ml_dtypes.float8_e4m3
RELU = mybir.ActivationFunctionType.Relu
COPY = mybir.ActivationFunctionType.Copy
DR = mybir.MatmulPerfMode.DoubleRow

S1 = 32.0                   # W1 fp8 scale (W1 ~ U(+-1/32))
S2 = 64.0                   # W2 fp8 scale (W2 ~ U(+-1/64))

# blob element offsets (per partition, fp8 elements)
OFF_QX = 0                              # [4, 2, 512]
OFF_XL = OFF_QX + J1 * 2 * BSH          # [4, 2, 512]
OFF_W1 = OFF_XL + J1 * 2 * BSH          # [32, 2, 4, 2, 128] (m, hl, j, pl, c)
W1_BLK = 2 * J1 * 2 * P                 # 2048 per m-tile
OFF_W2 = OFF_W1 + MT1 * W1_BLK          # [2, 2, 16, 2, 512] (n, hl, kk, pl, o)
W2_BLK = KK2 * 2 * NCH                  # 8192 per (n, hl) region
BLOB_EL = OFF_W2 + NT2 * 2 * W2_BLK     # 139264

WARMUP = 1                  # dummy matmuls to ramp PE p-state during DMA load
DEFER = 3                   # m-tiles by which GEMM1's xl-correction lags


def build_nc(reps=1):
    nc = bacc.Bacc("TRN2", target_bir_lowering=False, debug=False,
                   num_devices=N_CORES)

    blob = nc.declare_dram_parameter("blob", [P, BLOB_EL], F8, isOutput=False)
    b1t = nc.declare_dram_parameter("b1t", [P, MT1], F32, isOutput=False)
    b2r = nc.declare_dram_parameter("b2r", [P, N_OUT], F32, isOutput=False)
    y = nc.declare_dram_parameter("y", [BSH, N_OUT], BF16, isOutput=True)

    with tile.TileContext(nc) as tc:
        with (
            tc.tile_pool(name="const", bufs=1) as const,
            tc.tile_pool(name="inp", bufs=1) as inp,
            tc.tile_pool(name="hbuf", bufs=1) as hbuf,
            tc.tile_pool(name="hf", bufs=4) as hf_pool,
            tc.tile_pool(name="yout", bufs=4) as y_pool,
            tc.tile_pool(name="ps", bufs=6, space=bass.MemorySpace.PSUM) as ps_pool,
        ):
            # ---- PE warmup: ramp p-state while first DMAs stream ----
            dummy = const.tile([P, NCH], BF16)
            nc.vector.memset(dummy[:], 0.0)
            ps_w = ps_pool.tile([P, NCH], F32, tag="ps", name="ps_warm")
            for i in range(WARMUP):
                nc.tensor.matmul(ps_w[:, 0:P], dummy[:, 0:P], dummy[:, 0:P],
                                 start=(i == 0), stop=(i == WARMUP - 1))

            # ---- input tiles (slices of the blob, chunk-streamed) ----
            qx_sb = inp.tile([P, J1, 2, BSH], F8)
            xl_sb = inp.tile([P, J1, 2, BSH], F8)
            w1_sb = inp.tile([P, MT1, 2, J1, 2, P], F8)
            w2_sb = inp.tile([P, NT2, 2, KK2, 2, NCH], F8)
            b1_sb = const.tile([P, MT1], F32)
            b2_sb = const.tile([P, N_OUT], F32)

            hh_sb = hbuf.tile([P, KK2, 2, BSH], F8)
            hl_sb = hbuf.tile([P, KK2, 2, BSH], F8)

            # ---- DMA schedule (SP, consumption order) ----
            QXB = 2 * BSH  # one j-group of qx

            def w1_dma(m0, m1):
                nc.sync.dma_start(
                    out=w1_sb[:, m0:m1],
                    in_=blob[:, OFF_W1 + m0 * W1_BLK:OFF_W1 + m1 * W1_BLK],
                )

            nc.sync.dma_start(out=qx_sb[:], in_=blob[:, OFF_QX:OFF_XL])
            w1_dma(0, 1)
            w1_dma(1, 2)
            w1_dma(2, 3)
            nc.sync.dma_start(out=xl_sb[:, 0:2],
                              in_=blob[:, OFF_XL:OFF_XL + 2 * QXB])
            w1_dma(3, 4)
            nc.sync.dma_start(out=xl_sb[:, 2:4],
                              in_=blob[:, OFF_XL + 2 * QXB:OFF_XL + 4 * QXB])
            w1_dma(4, 5)
            nc.sync.dma_start(out=b1_sb[:], in_=b1t[:])
            for m in range(5, 10):
                w1_dma(m, m + 1)
            for m0 in range(10, 26, 4):
                w1_dma(m0, m0 + 4)
            w1_dma(26, 32)
            nc.sync.dma_start(out=b2_sb[:], in_=b2r[:])
            for n in range(NT2):
                for t in range(2):
                    off = OFF_W2 + (n * 2 + t) * W2_BLK
                    nc.sync.dma_start(
                        out=w2_sb[:, n, t],
                        in_=blob[:, off:off + W2_BLK],
                    )

            # Prime ACT/DVE with the small const-load waits so later evicts
            # stay under walrus' per-instruction sync-wait budget.
            prime1 = const.tile([P, 1], F32)
            nc.scalar.activation(prime1[:], b1_sb[:, 0:1], COPY)
            prime2 = const.tile([P, 1], F32)
            nc.vector.tensor_copy(prime2[:], b2_sb[:, 0:1])

            # ---- GEMM1: hT m-tiles; xl-correction (phase c) deferred by
            # DEFER tiles so the xl DMA streams under the hi-term compute ----
            ps_of = {}

            def g1_ab(m):
                ps = ps_pool.tile([P, BSH], F32, tag="ps", name=f"ps1_{m}")
                ps_of[m] = ps
                for j in range(J1):
                    nc.tensor.matmul(ps[:], w1_sb[:, m, 0, j], qx_sb[:, j],
                                     start=(j == 0), stop=False, perf_mode=DR)
                    nc.tensor.matmul(ps[:], w1_sb[:, m, 1, j], qx_sb[:, j],
                                     start=False, stop=False, perf_mode=DR)

            def g1_c(m):
                ps = ps_of.pop(m)
                for j in range(J1):
                    nc.tensor.matmul(ps[:], w1_sb[:, m, 0, j], xl_sb[:, j],
                                     start=False, stop=(j == J1 - 1),
                                     perf_mode=DR)
                h_f = hf_pool.tile([P, BSH], BF16, tag="hf", name="h_f")
                nc.scalar.activation(h_f[:], ps[:], RELU,
                                     bias=b1_sb[:, m:m + 1], scale=1.0 / S1)
                nc.vector.tensor_copy(hh_sb[:, m // 2, m % 2, :], h_f[:])
                nc.vector.tensor_sub(hl_sb[:, m // 2, m % 2, :], h_f[:],
                                     hh_sb[:, m // 2, m % 2, :])

            for m in range(MT1):
                g1_ab(m)
                if m >= DEFER:
                    g1_c(m - DEFER)
            for m in range(MT1 - DEFER, MT1):
                g1_c(m)

            # ---- GEMM2: bank-serial (n-major), k innermost.  Each bank's
            # PSUM is pre-seeded with 64*b2 by DVE (idle engine), so every
            # matmul accumulates (start=False) and the evict is a single
            # ACT copy with scale 1/64. ----
            def g2_bank(n, mb, c0, c1):
                # one psum accumulation over out columns [c0:c1) of chunk n
                w = c1 - c0
                ps2 = ps_pool.tile([P, w], F32, tag="ps", name="ps2")
                nc.vector.tensor_copy(
                    ps2[:], b2_sb[:, n * NCH + c0:n * NCH + c1])
                csl = slice(mb * P, (mb + 1) * P)
                for t, src_sb in ((0, hh_sb), (1, hh_sb), (2, hl_sb)):
                    wt = 1 if t == 1 else 0
                    for kk in range(KK2):
                        nc.tensor.matmul(
                            ps2[:], src_sb[:, kk, :, csl],
                            w2_sb[:, n, wt, kk, :, c0:c1],
                            start=False,
                            stop=(t == 2 and kk == KK2 - 1),
                            perf_mode=DR, skip_group_check=True)
                y_o = y_pool.tile([P, w], BF16, tag="yo", name="y_o")
                nc.scalar.activation(y_o[:], ps2[:], COPY, scale=1.0 / S2)
                nc.sync.dma_start(
                    out=y[mb * P:(mb + 1) * P,
                          n * NCH + c0:n * NCH + c1],
                    in_=y_o[:],
                )

            for n in range(NT2):
                for mb in range(MB2):
                    last_bank = (n == NT2 - 1 and mb == MB2 - 1)
                    if last_bank:
                        g2_bank(n, mb, 0, 384)
                        g2_bank(n, mb, 384, NCH)
                    else:
                        g2_bank(n, mb, 0, NCH)
    nc.compile()
    return nc


def _q8(a):
    return a.astype(E4)


def _q8f(a):
    return a.astype(E4).astype(np.float32)


def _prep_shared(W1, b1, W2, b2):
    """Quantize + pack the shared weight regions of the blob."""
    W1 = np.asarray(W1, dtype=np.float32)
    W2 = np.asarray(W2, dtype=np.float32)

    w1s = S1 * W1
    w1h = _q8f(w1s)
    w1l = _q8(w1s - w1h)
    w1h = _q8(w1s)

    def pack_w1(w):  # [1024, 4096] -> [p, m, j, pl, c]
        a = w.reshape(J1, 2, P, MT1, P)          # [j, pl, p, m, c]
        return np.ascontiguousarray(a.transpose(2, 3, 0, 1, 4))

    w1_region = np.stack([pack_w1(w1h), pack_w1(w1l)], axis=2)  # [p,m,hl,j,pl,c]
    w1_region = w1_region.reshape(P, MT1 * W1_BLK)

    w2s = S2 * W2
    w2h = _q8f(w2s)
    w2l = _q8(w2s - w2h)
    w2h = _q8(w2s)

    def pack_w2(w):  # [4096, 1024] -> [p, kk, pl, o]
        a = w.reshape(KK2, 2, P, N_OUT)          # [kk, pl, p, o]
        return np.ascontiguousarray(a.transpose(2, 0, 1, 3))

    w2hp, w2lp = pack_w2(w2h), pack_w2(w2l)      # [p, 16, 2, 1024]
    parts = []
    for n in range(NT2):
        for t in range(2):
            src = w2hp if t == 0 else w2lp
            parts.append(src[:, :, :, n * NCH:(n + 1) * NCH].reshape(P, W2_BLK))
    w2_region = np.concatenate(parts, axis=1)    # [p, 4*W2_BLK]

    b1t = np.ascontiguousarray(
        np.asarray(b1, dtype=np.float32).reshape(MT1, P).T
    )
    b2r = np.ascontiguousarray(
        np.broadcast_to(S2 * np.asarray(b2, dtype=np.float32), (P, N_OUT))
    )
    return w1_region, w2_region, b1t, b2r


def _pack_x(xs):
    """x shard [512, 1024] -> (qx, xl) packed [p, j, pl, n] regions."""
    x = np.asarray(xs, dtype=np.float32)
    qx = _q8f(x)
    xl = _q8(x - qx)
    qx = _q8(x)

    def pack(a):  # [512, 1024] -> [p, j*2*512]
        t = np.ascontiguousarray(a.T).reshape(J1, 2, P, BSH)  # [j, pl, p, n]
        return np.ascontiguousarray(t.transpose(2, 0, 1, 3)).reshape(P, J1 * 2 * BSH)

    return pack(qx), pack(xl)


def kernel(x, W1, b1, W2, b2):
    x = np.asarray(x, dtype=np.float32)
    w1_region, w2_region, b1t, b2r = _prep_shared(W1, b1, W2, b2)
    wtail = np.concatenate([w1_region, w2_region], axis=1)

    in_maps = []
    for i in range(N_CORES):
        qx, xl = _pack_x(x[i * BSH:(i + 1) * BSH, :])
        blob = np.concatenate([qx, xl, wtail], axis=1)
        assert blob.shape == (P, BLOB_EL), blob.shape
        in_maps.append({"blob": blob, "b1t": b1t, "b2r": b2r})

    nc = build_nc()
    res = run_bass_kernel_spmd(nc, in_maps, list(range(N_CORES)))
    y = np.concatenate(
        [np.asarray(res.results[i]["y"]) for i in range(N_CORES)], axis=0
    )
    return y.astype(np.float32)


if __name__ == "__main__":
    rng = np.random.default_rng(0)
    s1 = 1.0 / np.sqrt(N_IN)
    s2 = 1.0 / np.sqrt(N_HID)
    x = rng.standard_normal((B, N_IN), dtype=np.float32)
    W1 = rng.uniform(-s1, s1, (N_IN, N_HID)).astype(np.float32)
    b1 = rng.uniform(-s1, s1, N_HID).astype(np.float32)
    W2 = rng.uniform(-s2, s2, (N_HID, N_OUT)).astype(np.float32)
    b2 = rng.uniform(-s2, s2, N_OUT).astype(np.float32)
    y = kernel(x, W1, b1, W2, b2)
    h = np.maximum(x @ W1 + b1, 0)
    y_ref = h @ W2 + b2
    err = np.linalg.norm(y - y_ref) / np.linalg.norm(y_ref)
    print("rel_l2:", err)
